# revision 8
# baseline (speedup 1.0000x reference)
"""Multi-head attention (B=2, T=2048, D=512, H=8) on 8 trn2 NeuronCores.

Sharding: data + head parallel. Core c handles batch b = c//4 and head pair
p = c%4 (heads 2p, 2p+1 <-> feature rows 128p .. 128p+127 of the 512-wide
projection space).  Each core:
  - projects its 2 heads' q/k (layout [feat, tok], feat on partitions) and
    v (layout [tok, feat]) from host-pre-transposed bf16 inputs,
  - computes scoresT = k_h q_h^T in [key, query] orientation (keys on
    partitions) with row-tiled head pairs,
  - exp (scaled by 1/sqrt(dk), no max subtraction: |scores| <~ 10),
  - PV matmul with a ones column appended to v so the softmax denominator
    accumulates in psum row 64,
  - normalizes via reciprocal + ones-broadcast matmul,
  - projects through the core's Wo row-slice -> a [2048, 512] f32 partial.
Host sums the 4 partials per batch (the "all-reduce") and adds bo.
"""

import os
import sys

sys.path.insert(0, "/opt/trn_rl_repo")

from contextlib import ExitStack

import numpy as np
import ml_dtypes

import concourse.bass as bass
import concourse.tile as tile
from concourse import bacc, mybir
from concourse.bass_utils import run_bass_kernel_spmd

BF16 = mybir.dt.bfloat16
F32 = mybir.dt.float32

B, T, D = 2, 2048, 512
H, DK = 8, 64
N_CORES = 8
P = 128  # partitions / head-pair feature count
KC = D // P  # 4 contraction chunks of 128 over d_model
NKT = T // P  # 16 key tiles of 128
NQB = 4  # query blocks
QB = T // NQB  # 512 queries per block
QSUB = QB // P  # 4 psum sub-blocks of 128 queries


def _build_bass():
    nc = bacc.Bacc(trn_type="TRN2", num_devices=N_CORES, debug=False)

    qt_d = nc.dram_tensor("qt", [D, T], BF16, kind="ExternalInput").ap()
    kt_d = nc.dram_tensor("ktin", [D, T], BF16, kind="ExternalInput").ap()
    vt_d = nc.dram_tensor("vt", [D, T], BF16, kind="ExternalInput").ap()
    wqt_d = nc.dram_tensor("wqt", [D, P], BF16, kind="ExternalInput").ap()
    wkt_d = nc.dram_tensor("wkt", [D, P], BF16, kind="ExternalInput").ap()
    wvt_d = nc.dram_tensor("wvt", [D, P], BF16, kind="ExternalInput").ap()
    wota_d = nc.dram_tensor("wota", [DK, D], BF16, kind="ExternalInput").ap()
    wotb_d = nc.dram_tensor("wotb", [DK, D], BF16, kind="ExternalInput").ap()
    bq_d = nc.dram_tensor("bq", [P, 1], F32, kind="ExternalInput").ap()
    bk_d = nc.dram_tensor("bk", [P, 1], F32, kind="ExternalInput").ap()
    bv_d = nc.dram_tensor("bv", [1, P], F32, kind="ExternalInput").ap()
    out_d = nc.dram_tensor("outp", [T, D], F32, kind="ExternalOutput").ap()

    with tile.TileContext(nc) as tc, ExitStack() as ctx:
        singles = ctx.enter_context(tc.tile_pool(name="singles", bufs=1))
        qk_pool = ctx.enter_context(tc.tile_pool(name="qk", bufs=1))
        v_pool = ctx.enter_context(tc.tile_pool(name="vaug", bufs=NKT))
        exp_pool = ctx.enter_context(tc.tile_pool(name="exps", bufs=4))
        att_pool = ctx.enter_context(tc.tile_pool(name="att", bufs=3))
        rden_pool = ctx.enter_context(tc.tile_pool(name="rden", bufs=4))
        out_pool = ctx.enter_context(tc.tile_pool(name="outs", bufs=2))
        # PSUM: 2*2 (scores) + 2*1 (pv) + 2*1 (misc) = 8 banks
        ps_s = ctx.enter_context(tc.tile_pool(name="ps_s", bufs=2, space="PSUM"))
        ps_pv = ctx.enter_context(tc.tile_pool(name="ps_pv", bufs=2, space="PSUM"))
        ps_mi = ctx.enter_context(tc.tile_pool(name="ps_mi", bufs=2, space="PSUM"))

        # ---- load inputs (host pre-transposed / pre-cast) ----
        qt_sb = singles.tile([P, KC, T], BF16)
        nc.sync.dma_start(out=qt_sb, in_=qt_d.rearrange("(c p) t -> p c t", p=P))
        kt_sb = singles.tile([P, KC, T], BF16)
        nc.sync.dma_start(out=kt_sb, in_=kt_d.rearrange("(c p) t -> p c t", p=P))
        vt_sb = singles.tile([P, KC, T], BF16)
        nc.sync.dma_start(out=vt_sb, in_=vt_d.rearrange("(c p) t -> p c t", p=P))
        wqt_sb = singles.tile([P, KC, P], BF16)
        nc.sync.dma_start(out=wqt_sb, in_=wqt_d.rearrange("(c p) f -> p c f", p=P))
        wkt_sb = singles.tile([P, KC, P], BF16)
        nc.sync.dma_start(out=wkt_sb, in_=wkt_d.rearrange("(c p) f -> p c f", p=P))
        wvt_sb = singles.tile([P, KC, P], BF16)
        nc.sync.dma_start(out=wvt_sb, in_=wvt_d.rearrange("(c p) f -> p c f", p=P))
        wota_sb = singles.tile([DK, D], BF16)
        nc.sync.dma_start(out=wota_sb, in_=wota_d)
        wotb_sb = singles.tile([DK, D], BF16)
        nc.sync.dma_start(out=wotb_sb, in_=wotb_d)
        bq_sb = singles.tile([P, 1], F32)
        nc.sync.dma_start(out=bq_sb, in_=bq_d)
        bk_sb = singles.tile([P, 1], F32)
        nc.sync.dma_start(out=bk_sb, in_=bk_d)
        bv_sb = singles.tile([P, P], F32)
        nc.gpsimd.dma_start(
            out=bv_sb,
            in_=bass.AP(tensor=bv_d.tensor, offset=0, ap=[[0, P], [1, P]]),
        )
        ones_sb = singles.tile([1, DK], F32)
        nc.vector.memset(ones_sb, 1.0)

        # ---- projections ----
        # qT/kT: [128 feat, 2048 tok] bf16, feature rows = head pair
        qT = qk_pool.tile([P, T], BF16)
        kT = qk_pool.tile([P, T], BF16)
        for dst, src_sb, w_sb, b_sb in (
            (qT, qt_sb, wqt_sb, bq_sb),
            (kT, kt_sb, wkt_sb, bk_sb),
        ):
            for nq in range(KC):
                psq = ps_s.tile([P, 2, QB], F32, tag="scores")
                for kc in range(KC):
                    nc.tensor.matmul(
                        psq[:, 0, :],
                        w_sb[:, kc, :],
                        src_sb[:, kc, bass.ts(nq, QB)],
                        start=(kc == 0),
                        stop=(kc == KC - 1),
                    )
                nc.vector.tensor_copy(dst[:, bass.ts(nq, QB)], psq[:, 0, :])
                nc.vector.tensor_add(
                    dst[:, bass.ts(nq, QB)],
                    dst[:, bass.ts(nq, QB)],
                    b_sb[:, :].broadcast_to([P, QB]),
                )

        # v_aug tiles: [128 tok, 2, 65] bf16 (64 v features + ones column)
        v_aug = []
        for kt in range(NKT):
            psv = ps_pv.tile([P, P], F32, tag="pv")
            for kc in range(KC):
                nc.tensor.matmul(
                    psv,
                    vt_sb[:, kc, bass.ts(kt, P)],
                    wvt_sb[:, kc, :],
                    start=(kc == 0),
                    stop=(kc == KC - 1),
                )
            va = v_pool.tile([P, 2, DK + 1], BF16, tag="vaug")
            nc.vector.tensor_copy(va[:, 0, 0:DK], psv[:, 0:DK])
            nc.vector.tensor_copy(va[:, 1, 0:DK], psv[:, DK:P])
            nc.vector.tensor_add(va[:, 0, 0:DK], va[:, 0, 0:DK], bv_sb[:, 0:DK])
            nc.vector.tensor_add(va[:, 1, 0:DK], va[:, 1, 0:DK], bv_sb[:, DK:P])
            nc.vector.memset(va[:, :, DK : DK + 1], 1.0)
            v_aug.append(va)

        # ---- attention ----
        for qb in range(NQB):
            pv_a = ps_pv.tile([DK + 1, QB], F32, tag="pv")
            pv_b = ps_pv.tile([DK + 1, QB], F32, tag="pv")
            for kt in range(NKT):
                pss = ps_s.tile([P, 2, QB], F32, tag="scores")
                # row-tiled head pair: A on array rows 0-63, B on 64-127
                nc.tensor.matmul(
                    pss[:, 0, :],
                    kT[0:DK, bass.ts(kt, P)],
                    qT[0:DK, bass.ts(qb, QB)],
                    start=True,
                    stop=True,
                )
                nc.tensor.matmul(
                    pss[:, 1, :],
                    kT[DK:P, bass.ts(kt, P)],
                    qT[DK:P, bass.ts(qb, QB)],
                    start=True,
                    stop=True,
                )
                ex = exp_pool.tile([P, 2, QB], BF16, tag="exps")
                nc.scalar.activation(
                    out=ex,
                    in_=pss,
                    func=mybir.ActivationFunctionType.Exp,
                    scale=float(1.0 / np.sqrt(DK)),
                )
                nc.tensor.matmul(
                    pv_a,
                    v_aug[kt][:, 0, :],
                    ex[:, 0, :],
                    start=(kt == 0),
                    stop=(kt == NKT - 1),
                )
                nc.tensor.matmul(
                    pv_b,
                    v_aug[kt][:, 1, :],
                    ex[:, 1, :],
                    start=(kt == 0),
                    stop=(kt == NKT - 1),
                )

            # normalize:  att = pv[0:64] * (1/den) broadcast across partitions
            att_n = []
            for pv in (pv_a, pv_b):
                rden = rden_pool.tile([1, QB], F32, tag="rden")
                nc.vector.reciprocal(rden, pv[DK : DK + 1, :])
                bc = ps_mi.tile([DK, QB], F32, tag="misc")
                nc.tensor.matmul(bc, ones_sb, rden, start=True, stop=True)
                bc_sb = rden_pool.tile([DK, QB], F32, tag="bcsb")
                nc.vector.tensor_copy(bc_sb, bc)
                at = att_pool.tile([DK, QB], BF16, tag="att")
                nc.vector.tensor_mul(at, pv[0:DK, :], bc_sb)
                att_n.append(at)

            # Wo projection: accumulate both heads into one psum
            out_sb = out_pool.tile([P, QSUB, D], F32, tag="outs")
            for qs in range(QSUB):
                pso = ps_mi.tile([P, D], F32, tag="misc")
                nc.tensor.matmul(
                    pso, att_n[0][:, bass.ts(qs, P)], wota_sb, start=True, stop=False
                )
                nc.tensor.matmul(
                    pso, att_n[1][:, bass.ts(qs, P)], wotb_sb, start=False, stop=True
                )
                nc.vector.tensor_copy(out_sb[:, qs, :], pso)
            nc.sync.dma_start(
                out=out_d[bass.ts(qb, QB), :].rearrange("(c p) o -> p c o", p=P),
                in_=out_sb,
            )

    nc.compile()
    return nc


_NC_CACHE = None


def _get_nc():
    global _NC_CACHE
    if _NC_CACHE is None:
        _NC_CACHE = _build_bass()
    return _NC_CACHE


def _prep_in_maps(Q, K, V, Wq, bq, Wk, bk, Wv, bv, Wo, bo):
    bf = ml_dtypes.bfloat16
    f32 = np.float32
    qkvT = []  # per batch: transposed bf16 [D, T]
    for X in (Q, K, V):
        qkvT.append(
            [np.ascontiguousarray(X[b].T.astype(bf)) for b in range(B)]
        )
    woT = np.ascontiguousarray(Wo.T.astype(bf))  # [D feat, D out]
    in_maps = []
    for c in range(N_CORES):
        b, p = divmod(c, 4)
        rows = slice(P * p, P * (p + 1))
        in_maps.append(
            {
                "qt": qkvT[0][b],
                "ktin": qkvT[1][b],
                "vt": qkvT[2][b],
                "wqt": np.ascontiguousarray(Wq[rows].T.astype(bf)),
                "wkt": np.ascontiguousarray(Wk[rows].T.astype(bf)),
                "wvt": np.ascontiguousarray(Wv[rows].T.astype(bf)),
                "wota": np.ascontiguousarray(woT[P * p : P * p + DK]),
                "wotb": np.ascontiguousarray(woT[P * p + DK : P * (p + 1)]),
                "bq": np.ascontiguousarray(bq[rows].astype(f32).reshape(P, 1)),
                "bk": np.ascontiguousarray(bk[rows].astype(f32).reshape(P, 1)),
                "bv": np.ascontiguousarray(bv[rows].astype(f32).reshape(1, P)),
            }
        )
    return in_maps


def kernel(Q, K, V, Wq, bq, Wk, bk, Wv, bv, Wo, bo, _return_raw=False):
    nc = _get_nc()
    in_maps = _prep_in_maps(Q, K, V, Wq, bq, Wk, bk, Wv, bv, Wo, bo)
    res = run_bass_kernel_spmd(
        nc,
        in_maps,
        core_ids=list(range(N_CORES)),
        trace=os.environ.get("KERNEL_TRACE", "0") == "1",
    )
    parts = [r["outp"] for r in res.results]
    out = np.empty((B, T, D), np.float32)
    for b in range(B):
        out[b] = parts[4 * b]
        for p in range(1, 4):
            out[b] += parts[4 * b + p]
        out[b] += bo.astype(np.float32)
    if _return_raw:
        return out, res
    return out


# revision 9
# speedup vs baseline: 1.1405x; 1.1405x over previous
"""Multi-head attention (B=2, T=2048, D=512, H=8) on 8 trn2 NeuronCores.

Sharding: data + head parallel. Core c handles batch b = c//4 and head pair
p = c%4 (heads 2p, 2p+1 <-> feature rows 128p .. 128p+127 of the 512-wide
projection space).  Each core:
  - projects its 2 heads' q/k (layout [feat, tok], feat on partitions) and
    v (layout [tok, feat]) from host-pre-transposed bf16 inputs,
  - computes scoresT = k_h q_h^T in [key, query] orientation (keys on
    partitions) with row-tiled head pairs,
  - exp (scaled by 1/sqrt(dk), no max subtraction: |scores| <~ 10),
  - PV matmul with a ones column appended to v so the softmax denominator
    accumulates in psum row 64,
  - normalizes via reciprocal + gpsimd partition_broadcast,
  - projects through the core's Wo row-slice -> a [2048, 512] f32 partial.
Host sums the 4 partials per batch (the "all-reduce") and adds bo.
"""

import os
import sys

sys.path.insert(0, "/opt/trn_rl_repo")

from contextlib import ExitStack

import numpy as np
import ml_dtypes

import concourse.bass as bass
import concourse.bass_isa as bass_isa
import concourse.tile as tile
from concourse import bacc, mybir
from concourse.bass_utils import run_bass_kernel_spmd

BF16 = mybir.dt.bfloat16
F32 = mybir.dt.float32

B, T, D = 2, 2048, 512
H, DK = 8, 64
N_CORES = 8
P = 128  # partitions / head-pair feature count
KC = D // P  # 4 contraction chunks of 128 over d_model
NKT = T // P  # 16 key tiles of 128
NQB = 4  # query blocks
QB = T // NQB  # 512 queries per block
QSUB = QB // P  # 4 psum sub-blocks of 128 queries
NTC = 4  # token chunks for pipelined loads/projections (512 tokens each)


def _build_bass(with_bias):
    nc = bacc.Bacc(trn_type="TRN2", num_devices=N_CORES, debug=False)

    qt_d = nc.dram_tensor("qt", [D, T], BF16, kind="ExternalInput").ap()
    kt_d = nc.dram_tensor("ktin", [D, T], BF16, kind="ExternalInput").ap()
    vt_d = nc.dram_tensor("vt", [D, T], BF16, kind="ExternalInput").ap()
    wqt_d = nc.dram_tensor("wqt", [D, P], BF16, kind="ExternalInput").ap()
    wkt_d = nc.dram_tensor("wkt", [D, P], BF16, kind="ExternalInput").ap()
    wvt_d = nc.dram_tensor("wvt", [D, P], BF16, kind="ExternalInput").ap()
    wota_d = nc.dram_tensor("wota", [DK, D], BF16, kind="ExternalInput").ap()
    wotb_d = nc.dram_tensor("wotb", [DK, D], BF16, kind="ExternalInput").ap()
    if with_bias:
        bq_d = nc.dram_tensor("bq", [P, 1], F32, kind="ExternalInput").ap()
        bk_d = nc.dram_tensor("bk", [P, 1], F32, kind="ExternalInput").ap()
        bv_d = nc.dram_tensor("bv", [1, P], F32, kind="ExternalInput").ap()
    out_d = nc.dram_tensor("outp", [T, D], F32, kind="ExternalOutput").ap()

    with tile.TileContext(nc) as tc, ExitStack() as ctx:
        singles = ctx.enter_context(tc.tile_pool(name="singles", bufs=1))
        qk_pool = ctx.enter_context(tc.tile_pool(name="qk", bufs=1))
        v_pool = ctx.enter_context(tc.tile_pool(name="vaug", bufs=NKT))
        exp_pool = ctx.enter_context(tc.tile_pool(name="exps", bufs=4))
        att_pool = ctx.enter_context(tc.tile_pool(name="att", bufs=3))
        rden_pool = ctx.enter_context(tc.tile_pool(name="rden", bufs=4))
        out_pool = ctx.enter_context(tc.tile_pool(name="outs", bufs=2))
        # PSUM: 2*2 (scores) + 2*1 (pv) + 2*1 (out) = 8 banks
        ps_s = ctx.enter_context(tc.tile_pool(name="ps_s", bufs=2, space="PSUM"))
        ps_pv = ctx.enter_context(tc.tile_pool(name="ps_pv", bufs=2, space="PSUM"))
        ps_mi = ctx.enter_context(tc.tile_pool(name="ps_mi", bufs=2, space="PSUM"))

        # ---- weight/bias loads ----
        wqt_sb = singles.tile([P, KC, P], BF16)
        nc.sync.dma_start(out=wqt_sb, in_=wqt_d.rearrange("(c p) f -> p c f", p=P))
        wkt_sb = singles.tile([P, KC, P], BF16)
        nc.sync.dma_start(out=wkt_sb, in_=wkt_d.rearrange("(c p) f -> p c f", p=P))
        wvt_sb = singles.tile([P, KC, P], BF16)
        nc.sync.dma_start(out=wvt_sb, in_=wvt_d.rearrange("(c p) f -> p c f", p=P))
        wota_sb = singles.tile([DK, D], BF16)
        nc.sync.dma_start(out=wota_sb, in_=wota_d)
        wotb_sb = singles.tile([DK, D], BF16)
        nc.sync.dma_start(out=wotb_sb, in_=wotb_d)
        if with_bias:
            bq_sb = singles.tile([P, 1], F32)
            nc.sync.dma_start(out=bq_sb, in_=bq_d)
            bk_sb = singles.tile([P, 1], F32)
            nc.sync.dma_start(out=bk_sb, in_=bk_d)
            bv_sb = singles.tile([P, P], F32)
            nc.gpsimd.dma_start(
                out=bv_sb,
                in_=bass.AP(tensor=bv_d.tensor, offset=0, ap=[[0, P], [1, P]]),
            )

        # ---- chunked input loads (512-token slices) ----
        qt_sb = singles.tile([P, KC, T], BF16)
        kt_sb = singles.tile([P, KC, T], BF16)
        vt_sb = singles.tile([P, KC, T], BF16)
        for c in range(NTC):
            sl = bass.ts(c, T // NTC)
            nc.sync.dma_start(
                out=kt_sb[:, :, sl],
                in_=kt_d.rearrange("(c p) t -> p c t", p=P)[:, :, sl],
            )
            nc.sync.dma_start(
                out=qt_sb[:, :, sl],
                in_=qt_d.rearrange("(c p) t -> p c t", p=P)[:, :, sl],
            )
            nc.sync.dma_start(
                out=vt_sb[:, :, sl],
                in_=vt_d.rearrange("(c p) t -> p c t", p=P)[:, :, sl],
            )

        # ---- projections (per 512-token chunk) ----
        qT = qk_pool.tile([P, T], BF16)
        kT = qk_pool.tile([P, T], BF16)
        v_aug = [None] * NKT
        for c in range(NTC):
            sl = bass.ts(c, T // NTC)
            for dst, src_sb, w_sb, bname in (
                (kT, kt_sb, wkt_sb, "bk"),
                (qT, qt_sb, wqt_sb, "bq"),
            ):
                psq = ps_s.tile([P, 2, QB], F32, tag="scores")
                for kc in range(KC):
                    nc.tensor.matmul(
                        psq[:, 0, :],
                        w_sb[:, kc, :],
                        src_sb[:, kc, sl],
                        start=(kc == 0),
                        stop=(kc == KC - 1),
                    )
                nc.vector.tensor_copy(dst[:, sl], psq[:, 0, :])
                if with_bias:
                    b_sb = bq_sb if bname == "bq" else bk_sb
                    nc.vector.tensor_add(
                        dst[:, sl], dst[:, sl], b_sb[:, :].broadcast_to([P, QB])
                    )
            for kt in range(c * NKT // NTC, (c + 1) * NKT // NTC):
                psv = ps_pv.tile([P, P], F32, tag="pv")
                for kc in range(KC):
                    nc.tensor.matmul(
                        psv,
                        vt_sb[:, kc, bass.ts(kt, P)],
                        wvt_sb[:, kc, :],
                        start=(kc == 0),
                        stop=(kc == KC - 1),
                    )
                va = v_pool.tile([P, 2, DK + 1], BF16, tag="vaug")
                # both heads in one strided copy: psum f -> (head, f)
                nc.vector.tensor_copy(
                    va[:, :, 0:DK], psv[:, :].rearrange("p (h f) -> p h f", h=2)
                )
                if with_bias:
                    nc.vector.tensor_add(
                        va[:, :, 0:DK],
                        va[:, :, 0:DK],
                        bv_sb[:, :].rearrange("p (h f) -> p h f", h=2),
                    )
                nc.vector.memset(va[:, :, DK : DK + 1], 1.0)
                v_aug[kt] = va

        # ---- attention ----
        inv_sqrt_dk = float(1.0 / np.sqrt(DK))
        for qb in range(NQB):
            pv_a = ps_pv.tile([DK + 1, QB], F32, tag="pv")
            pv_b = ps_pv.tile([DK + 1, QB], F32, tag="pv")
            for kt in range(NKT):
                pss = ps_s.tile([P, 2, QB], F32, tag="scores")
                # row-tiled head pair: A on array rows 0-63, B on 64-127
                nc.tensor.matmul(
                    pss[:, 0, :],
                    kT[0:DK, bass.ts(kt, P)],
                    qT[0:DK, bass.ts(qb, QB)],
                    start=True,
                    stop=True,
                )
                nc.tensor.matmul(
                    pss[:, 1, :],
                    kT[DK:P, bass.ts(kt, P)],
                    qT[DK:P, bass.ts(qb, QB)],
                    start=True,
                    stop=True,
                )
                ex = exp_pool.tile([P, 2, QB], BF16, tag="exps")
                nc.scalar.activation(
                    out=ex,
                    in_=pss,
                    func=mybir.ActivationFunctionType.Exp,
                    scale=inv_sqrt_dk,
                )
                nc.tensor.matmul(
                    pv_a,
                    v_aug[kt][:, 0, :],
                    ex[:, 0, :],
                    start=(kt == 0),
                    stop=(kt == NKT - 1),
                )
                nc.tensor.matmul(
                    pv_b,
                    v_aug[kt][:, 1, :],
                    ex[:, 1, :],
                    start=(kt == 0),
                    stop=(kt == NKT - 1),
                )

            # normalize:  att = pv[0:64] * (1/den) broadcast across partitions
            att_n = []
            for pv in (pv_a, pv_b):
                rden = rden_pool.tile([1, QB], F32, tag="rden")
                nc.vector.reciprocal(rden, pv[DK : DK + 1, :])
                rbc = rden_pool.tile([DK, QB], F32, tag="rbc")
                nc.gpsimd.partition_broadcast(rbc, rden, channels=DK)
                at = att_pool.tile([DK, QB], BF16, tag="att")
                nc.vector.tensor_mul(at, pv[0:DK, :], rbc)
                att_n.append(at)

            # Wo projection: accumulate both heads into one psum
            out_sb = out_pool.tile([P, QSUB, D], F32, tag="outs")
            for qs in range(QSUB):
                pso = ps_mi.tile([P, D], F32, tag="misc")
                nc.tensor.matmul(
                    pso, att_n[0][:, bass.ts(qs, P)], wota_sb, start=True, stop=False
                )
                nc.tensor.matmul(
                    pso, att_n[1][:, bass.ts(qs, P)], wotb_sb, start=False, stop=True
                )
                nc.vector.tensor_copy(out_sb[:, qs, :], pso)
            nc.sync.dma_start(
                out=out_d[bass.ts(qb, QB), :].rearrange("(c p) o -> p c o", p=P),
                in_=out_sb,
            )

    nc.compile()
    return nc


_NC_CACHE = {}


def _get_nc(with_bias):
    if with_bias not in _NC_CACHE:
        _NC_CACHE[with_bias] = _build_bass(with_bias)
    return _NC_CACHE[with_bias]


def _prep_in_maps(Q, K, V, Wq, bq, Wk, bk, Wv, bv, Wo, bo, with_bias):
    bf = ml_dtypes.bfloat16
    f32 = np.float32
    qkvT = []  # per batch: transposed bf16 [D, T]
    for X in (Q, K, V):
        qkvT.append([np.ascontiguousarray(X[b].T.astype(bf)) for b in range(B)])
    woT = np.ascontiguousarray(Wo.T.astype(bf))  # [D feat, D out]
    in_maps = []
    for c in range(N_CORES):
        b, p = divmod(c, 4)
        rows = slice(P * p, P * (p + 1))
        m = {
            "qt": qkvT[0][b],
            "ktin": qkvT[1][b],
            "vt": qkvT[2][b],
            "wqt": np.ascontiguousarray(Wq[rows].T.astype(bf)),
            "wkt": np.ascontiguousarray(Wk[rows].T.astype(bf)),
            "wvt": np.ascontiguousarray(Wv[rows].T.astype(bf)),
            "wota": np.ascontiguousarray(woT[P * p : P * p + DK]),
            "wotb": np.ascontiguousarray(woT[P * p + DK : P * (p + 1)]),
        }
        if with_bias:
            m["bq"] = np.ascontiguousarray(bq[rows].astype(f32).reshape(P, 1))
            m["bk"] = np.ascontiguousarray(bk[rows].astype(f32).reshape(P, 1))
            m["bv"] = np.ascontiguousarray(bv[rows].astype(f32).reshape(1, P))
        in_maps.append(m)
    return in_maps


def kernel(Q, K, V, Wq, bq, Wk, bk, Wv, bv, Wo, bo, _return_raw=False):
    with_bias = bool(np.any(bq) or np.any(bk) or np.any(bv))
    nc = _get_nc(with_bias)
    in_maps = _prep_in_maps(Q, K, V, Wq, bq, Wk, bk, Wv, bv, Wo, bo, with_bias)
    res = run_bass_kernel_spmd(
        nc,
        in_maps,
        core_ids=list(range(N_CORES)),
        trace=os.environ.get("KERNEL_TRACE", "0") == "1",
    )
    parts = [r["outp"] for r in res.results]
    out = np.empty((B, T, D), np.float32)
    for b in range(B):
        out[b] = parts[4 * b]
        for p in range(1, 4):
            out[b] += parts[4 * b + p]
        out[b] += bo.astype(np.float32)
    if _return_raw:
        return out, res
    return out


# revision 66
# speedup vs baseline: 31939.3686x; 28003.9531x over previous
"""Multi-head attention (B=2, T=2048, D=512, H=8) on 8 trn2 NeuronCores.

Sharding: data + head parallel. Core c handles batch b = c//4 and head pair
p = c%4 (heads 2p, 2p+1 <-> feature rows 128p .. 128p+127 of the 512-wide
projection space).  Each core:
  - projects its 2 heads' q/k (layout [feat, tok], feat on partitions) and
    v (layout [tok, feat]) from host-pre-transposed bf16 inputs,
  - computes scoresT = k_h q_h^T in [key, query] orientation (keys on
    partitions) with row-tiled head pairs,
  - exp (scaled by 1/sqrt(dk), no max subtraction: |scores| <~ 10),
  - PV matmul with a ones column appended to v so the softmax denominator
    accumulates in psum row 64,
  - normalizes via reciprocal + gpsimd partition_broadcast,
  - projects through the core's Wo row-slice -> a [2048, 512] f32 partial.
Host sums the 4 partials per batch (the "all-reduce") and adds bo.
"""

import os
import sys

sys.path.insert(0, "/opt/trn_rl_repo")

from contextlib import ExitStack

import numpy as np
import ml_dtypes

import concourse.bass as bass
import concourse.tile as tile
from concourse import bacc, mybir
from concourse.bass_utils import run_bass_kernel_spmd

BF16 = mybir.dt.bfloat16
F32 = mybir.dt.float32

B, T, D = 2, 2048, 512
H, DK = 8, 64
N_CORES = 8
P = 128  # partitions / head-pair feature count
KC = D // P  # 4 contraction chunks of 128 over d_model
NKT = T // P  # 16 key tiles of 128
NQB = 4  # query blocks
QB = T // NQB  # 512 queries per block
QSUB = QB // P  # 4 psum sub-blocks of 128 queries
# Token chunks for pipelined loads/projections.  Must equal NQB: each query
# block's qT must be fully projected by one chunk's q-projection before the
# block's first QK reads it (trace order implies no read-before-write).
NTC = 4

# pool-size knobs (tuned via cost-model sim in analyze.py)
CFG = {
    "exp_bufs": 6,
    "att_bufs": 3,
    "out_bufs": 2,
    "ps_s_bufs": 2,
    "ps_pv_bufs": 2,
    "ps_mi_bufs": 2,
}


def _build_bass(with_bias):
    nc = bacc.Bacc(trn_type="TRN2", num_devices=N_CORES, debug=False)

    qt_d = nc.dram_tensor("qt", [D, T], BF16, kind="ExternalInput").ap()
    kt_d = nc.dram_tensor("ktin", [D, T], BF16, kind="ExternalInput").ap()
    vt_d = nc.dram_tensor("vt", [D, T], BF16, kind="ExternalInput").ap()
    # q/k/v weights arrive host-pre-swizzled as one [p, 3, c, f]
    # (partition-major) tensor so a single contiguous DMA loads all three
    wqkv_d = nc.dram_tensor("wqkv", [P, 3, KC, P], BF16, kind="ExternalInput").ap()
    wota_d = nc.dram_tensor("wota", [DK, D], BF16, kind="ExternalInput").ap()
    wotb_d = nc.dram_tensor("wotb", [DK, D], BF16, kind="ExternalInput").ap()
    if with_bias:
        bq_d = nc.dram_tensor("bq", [P, 1], F32, kind="ExternalInput").ap()
        bk_d = nc.dram_tensor("bk", [P, 1], F32, kind="ExternalInput").ap()
        bv_d = nc.dram_tensor("bv", [1, P], F32, kind="ExternalInput").ap()
    out_d = nc.dram_tensor("outp", [T, D], F32, kind="ExternalOutput").ap()

    with tile.TileContext(nc) as tc, ExitStack() as ctx:
        singles = ctx.enter_context(tc.tile_pool(name="singles", bufs=1))
        qk_pool = ctx.enter_context(tc.tile_pool(name="qk", bufs=1))
        v_pool = ctx.enter_context(tc.tile_pool(name="vaug", bufs=NKT))
        exp_pool = ctx.enter_context(tc.tile_pool(name="exps", bufs=CFG["exp_bufs"]))
        att_pool = ctx.enter_context(tc.tile_pool(name="att", bufs=CFG["att_bufs"]))
        rden_pool = ctx.enter_context(tc.tile_pool(name="rden", bufs=4))
        out_pool = ctx.enter_context(tc.tile_pool(name="outs", bufs=CFG["out_bufs"]))
        # PSUM: scores 2*2 + pv 2*1 + misc 2*1 = 8 banks
        ps_s = ctx.enter_context(
            tc.tile_pool(name="ps_s", bufs=CFG["ps_s_bufs"], space="PSUM")
        )
        ps_pv = ctx.enter_context(
            tc.tile_pool(name="ps_pv", bufs=CFG["ps_pv_bufs"], space="PSUM")
        )
        ps_mi = ctx.enter_context(
            tc.tile_pool(name="ps_mi", bufs=CFG["ps_mi_bufs"], space="PSUM")
        )

        # ---- weight/bias loads ----
        wqkv_sb = singles.tile([P, 3, KC, P], BF16)
        nc.sync.dma_start(out=wqkv_sb, in_=wqkv_d)
        wqt_sb = wqkv_sb[:, 0]
        wkt_sb = wqkv_sb[:, 1]
        wvt_sb = wqkv_sb[:, 2]
        if with_bias:
            bq_sb = singles.tile([P, 1], F32)
            nc.sync.dma_start(out=bq_sb, in_=bq_d)
            bk_sb = singles.tile([P, 1], F32)
            nc.sync.dma_start(out=bk_sb, in_=bk_d)
            bv_sb = singles.tile([P, P], F32)
            nc.gpsimd.dma_start(
                out=bv_sb,
                in_=bass.AP(tensor=bv_d.tensor, offset=0, ap=[[0, P], [1, P]]),
            )

        # ---- chunked input loads (512-token slices) ----
        qt_sb = singles.tile([P, KC, T], BF16)
        kt_sb = singles.tile([P, KC, T], BF16)
        vt_sb = singles.tile([P, KC, T], BF16)
        # qb0 only needs QT chunk 0; all of KT/VT gate qb0's PV chain,
        # so load those first and defer QT chunks 1-3.
        # First K slice is only 128 tokens (one k-tile) so the first
        # QK->exp fires as soon as possible.
        ktr = kt_d.rearrange("(c p) t -> p c t", p=P)
        qtr = qt_d.rearrange("(c p) t -> p c t", p=P)
        vtr = vt_d.rearrange("(c p) t -> p c t", p=P)
        nc.sync.dma_start(out=kt_sb[:, :, 0:P], in_=ktr[:, :, 0:P])
        nc.sync.dma_start(out=qt_sb[:, :, 0 : T // NTC], in_=qtr[:, :, 0 : T // NTC])
        nc.sync.dma_start(
            out=kt_sb[:, :, P : T // NTC], in_=ktr[:, :, P : T // NTC]
        )
        # KT chunk c+1 is prefetched ahead of QT/VT chunk c: K gates the
        # QK->exp critical path while V only feeds the lagging PV chain.
        for c in range(1, NTC):
            sl = bass.ts(c, T // NTC)
            nc.sync.dma_start(out=kt_sb[:, :, sl], in_=ktr[:, :, sl])
            slp = bass.ts(c - 1, T // NTC)
            if c >= 2:
                nc.sync.dma_start(out=qt_sb[:, :, slp], in_=qtr[:, :, slp])
            nc.sync.dma_start(out=vt_sb[:, :, slp], in_=vtr[:, :, slp])
        slz = bass.ts(NTC - 1, T // NTC)
        nc.sync.dma_start(out=qt_sb[:, :, slz], in_=qtr[:, :, slz])
        nc.sync.dma_start(out=vt_sb[:, :, slz], in_=vtr[:, :, slz])
        # Wo slices are not needed until the first qb tail — load last.
        wota_sb = singles.tile([DK, D], BF16)
        nc.sync.dma_start(out=wota_sb, in_=wota_d)
        wotb_sb = singles.tile([DK, D], BF16)
        nc.sync.dma_start(out=wotb_sb, in_=wotb_d)

        # ---- projections + attention ----
        qT = qk_pool.tile([P, T], BF16)
        kT = qk_pool.tile([P, T], BF16)
        v_aug = [None] * NKT
        inv_sqrt_dk = float(1.0 / np.sqrt(DK))

        def emit_qk_proj(dst, src_sb, w_sb, b_sb, c, lo=None, cs=None):
            if cs is None:
                cs = T // NTC
            sl = bass.ds(c * (T // NTC) if lo is None else lo, cs)
            psq = ps_mi.tile([P, QB], F32, tag="misc")
            for kc in range(KC):
                nc.tensor.matmul(
                    psq[:, 0:cs],
                    w_sb[:, kc, :],
                    src_sb[:, kc, sl],
                    start=(kc == 0),
                    stop=(kc == KC - 1),
                )
            nc.vector.tensor_copy(dst[:, sl], psq[:, 0:cs])
            if b_sb is not None:
                nc.vector.tensor_add(
                    dst[:, sl], dst[:, sl], b_sb[:, :].broadcast_to([P, cs])
                )

        def emit_v_proj(kt):
            psv = ps_mi.tile([P, P], F32, tag="misc")
            for kc in range(KC):
                nc.tensor.matmul(
                    psv,
                    vt_sb[:, kc, bass.ts(kt, P)],
                    wvt_sb[:, kc, :],
                    start=(kc == 0),
                    stop=(kc == KC - 1),
                )
            va = v_pool.tile([P, 2, DK + 1], BF16, tag="vaug")
            # both heads in one strided copy: psum f -> (head, f)
            nc.vector.tensor_copy(
                va[:, :, 0:DK], psv[:, :].rearrange("p (h f) -> p h f", h=2)
            )
            if with_bias:
                nc.vector.tensor_add(
                    va[:, :, 0:DK],
                    va[:, :, 0:DK],
                    bv_sb[:, :].rearrange("p (h f) -> p h f", h=2),
                )
            nc.vector.memset(va[:, :, DK : DK + 1], 1.0)
            v_aug[kt] = va

        def emit_pv(pv_a, pv_b, kt, ex):
            nc.tensor.matmul(
                pv_a,
                v_aug[kt][:, 0, :],
                ex[:, 0, :],
                start=(kt == 0),
                stop=(kt == NKT - 1),
            )
            nc.tensor.matmul(
                pv_b,
                v_aug[kt][:, 1, :],
                ex[:, 1, :],
                start=(kt == 0),
                stop=(kt == NKT - 1),
            )

        def emit_attn_group(qb, pv_a, pv_b, kts, pending, lag=True):
            # software-pipelined: PV(kt) is emitted AFTER QK/exp(kt+1) so the
            # scalar engine (the bottleneck) is never starved of fresh scores.
            for kt in kts:
                pss = ps_s.tile([P, 2, QB], F32, tag="scores")
                # row-tiled head pair: A on array rows 0-63, B on 64-127
                nc.tensor.matmul(
                    pss[:, 0, :],
                    kT[0:DK, bass.ts(kt, P)],
                    qT[0:DK, bass.ts(qb, QB)],
                    start=True,
                    stop=True,
                )
                nc.tensor.matmul(
                    pss[:, 1, :],
                    kT[DK:P, bass.ts(kt, P)],
                    qT[DK:P, bass.ts(qb, QB)],
                    start=True,
                    stop=True,
                )
                ex = exp_pool.tile([P, 2, QB], BF16, tag="exps")
                nc.scalar.activation(
                    out=ex,
                    in_=pss,
                    func=mybir.ActivationFunctionType.Exp,
                    scale=inv_sqrt_dk,
                )
                if lag:
                    if pending:
                        emit_pv(pv_a, pv_b, *pending.pop())
                    pending.append((kt, ex))
                else:
                    emit_pv(pv_a, pv_b, kt, ex)

        def emit_qb_tail(qb, pv_a, pv_b):
            # normalize:  att = pv[0:64] * (1/den) broadcast across partitions
            att_n = []
            for pv in (pv_a, pv_b):
                rden = rden_pool.tile([1, QB], F32, tag="rden")
                nc.vector.reciprocal(rden, pv[DK : DK + 1, :])
                rbc = rden_pool.tile([DK, QB], F32, tag="rbc")
                nc.gpsimd.partition_broadcast(rbc, rden, channels=DK)
                at = att_pool.tile([DK, QB], BF16, tag="att")
                nc.vector.tensor_mul(at, pv[0:DK, :], rbc)
                att_n.append(at)

            # Wo projection: accumulate both heads into one psum.  The last
            # qb streams per-qsub DMAs (shortest path to the final byte) and
            # alternates psum->sbuf copies between ACT (idle by then) and DVE.
            last = qb == NQB - 1
            nhalf = 2
            per = QSUB // nhalf
            for half in range(nhalf):
                out_sb = out_pool.tile([P, per, D], F32, tag="outs")
                for i in range(per):
                    qs = half * per + i
                    pso = ps_mi.tile([P, D], F32, tag="misc")
                    nc.tensor.matmul(
                        pso,
                        att_n[0][:, bass.ts(qs, P)],
                        wota_sb,
                        start=True,
                        stop=False,
                    )
                    nc.tensor.matmul(
                        pso,
                        att_n[1][:, bass.ts(qs, P)],
                        wotb_sb,
                        start=False,
                        stop=True,
                    )
                    if last and qs % 2 == 0:
                        nc.scalar.copy(out_sb[:, i, :], pso)
                    else:
                        nc.vector.tensor_copy(out_sb[:, i, :], pso)
                nc.sync.dma_start(
                    out=out_d[bass.ds(qb * QB + half * per * P, per * P), :].rearrange(
                        "(c p) o -> p c o", p=P
                    ),
                    in_=out_sb,
                )

        # qb0 is interleaved with the per-chunk projections so the scalar
        # engine (softmax exp — the bottleneck) starts as early as possible.
        bqs = bq_sb if with_bias else None
        bks = bk_sb if with_bias else None
        pv0_a = ps_pv.tile([DK + 1, QB], F32, tag="pv")
        pv0_b = ps_pv.tile([DK + 1, QB], F32, tag="pv")
        kpc = NKT // NTC  # k-tiles per chunk
        pending = []
        emit_qk_proj(kT, kt_sb, wkt_sb, bks, 0, lo=0, cs=P)
        emit_qk_proj(qT, qt_sb, wqt_sb, bqs, 0)
        emit_attn_group(0, pv0_a, pv0_b, [0], pending)
        emit_qk_proj(kT, kt_sb, wkt_sb, bks, 0, lo=P, cs=T // NTC - P)
        for c in range(NTC):
            # next chunk's k/q projections are emitted BEFORE this chunk's
            # v-projections so their psum allocations are FIFO-ahead of the
            # (VT-gated) psv tiles in the shared misc pool.
            if c + 1 < NTC:
                emit_qk_proj(kT, kt_sb, wkt_sb, bks, c + 1)
            for kt in range(c * kpc, (c + 1) * kpc):
                emit_v_proj(kt)
            lo = 1 if c == 0 else c * kpc
            emit_attn_group(0, pv0_a, pv0_b, range(lo, (c + 1) * kpc), pending)
            if c + 1 < NTC:
                emit_qk_proj(qT, qt_sb, wqt_sb, bqs, c + 1)
        if pending:
            emit_pv(pv0_a, pv0_b, *pending.pop())

        # Tails are emitted one qb late so the previous qb's Wo/copies have
        # LOWER scheduler priority than the current qb's QK matmuls feeding
        # the scalar engine.  The final qb gets no PV lag and an immediate
        # tail (it's the critical path to the last output byte).
        prev = (0, pv0_a, pv0_b)
        for qb in range(1, NQB):
            pv_a = ps_pv.tile([DK + 1, QB], F32, tag="pv")
            pv_b = ps_pv.tile([DK + 1, QB], F32, tag="pv")
            pending = []
            last = qb == NQB - 1
            emit_attn_group(qb, pv_a, pv_b, range(NKT), pending, lag=not last)
            if pending:
                emit_pv(pv_a, pv_b, *pending.pop())
            emit_qb_tail(*prev)
            prev = (qb, pv_a, pv_b)
        emit_qb_tail(*prev)

    nc.compile()
    return nc


_NC_CACHE = {}


def _get_nc(with_bias):
    if with_bias not in _NC_CACHE:
        _NC_CACHE[with_bias] = _build_bass(with_bias)
    return _NC_CACHE[with_bias]


def _prep_in_maps(Q, K, V, Wq, bq, Wk, bk, Wv, bv, Wo, bo, with_bias):
    bf = ml_dtypes.bfloat16
    f32 = np.float32
    qkvT = []  # per batch: transposed bf16 [D, T]
    for X in (Q, K, V):
        qkvT.append([np.ascontiguousarray(X[b].T.astype(bf)) for b in range(B)])
    woT = np.ascontiguousarray(Wo.T.astype(bf))  # [D feat, D out]

    def swz(w_rows):  # [P, D] slice of W -> transposed+partition-major [P, KC, P]
        return np.ascontiguousarray(
            w_rows.T.astype(bf).reshape(KC, P, P).transpose(1, 0, 2)
        )
    in_maps = []
    for c in range(N_CORES):
        b, p = divmod(c, 4)
        rows = slice(P * p, P * (p + 1))
        m = {
            "qt": qkvT[0][b],
            "ktin": qkvT[1][b],
            "vt": qkvT[2][b],
            "wqkv": np.ascontiguousarray(
                np.stack([swz(Wq[rows]), swz(Wk[rows]), swz(Wv[rows])], axis=1)
            ),
            "wota": np.ascontiguousarray(woT[P * p : P * p + DK]),
            "wotb": np.ascontiguousarray(woT[P * p + DK : P * (p + 1)]),
        }
        if with_bias:
            m["bq"] = np.ascontiguousarray(bq[rows].astype(f32).reshape(P, 1))
            m["bk"] = np.ascontiguousarray(bk[rows].astype(f32).reshape(P, 1))
            m["bv"] = np.ascontiguousarray(bv[rows].astype(f32).reshape(1, P))
        in_maps.append(m)
    return in_maps


def kernel(Q, K, V, Wq, bq, Wk, bk, Wv, bv, Wo, bo, _return_raw=False):
    # accept jax arrays / lists transparently
    Q, K, V = np.asarray(Q), np.asarray(K), np.asarray(V)
    Wq, Wk, Wv, Wo = (np.asarray(x) for x in (Wq, Wk, Wv, Wo))
    bq, bk, bv, bo = (np.asarray(x) for x in (bq, bk, bv, bo))
    with_bias = bool(np.any(bq) or np.any(bk) or np.any(bv))
    nc = _get_nc(with_bias)
    in_maps = _prep_in_maps(Q, K, V, Wq, bq, Wk, bk, Wv, bv, Wo, bo, with_bias)
    res = run_bass_kernel_spmd(
        nc,
        in_maps,
        core_ids=list(range(N_CORES)),
        trace=os.environ.get("KERNEL_TRACE", "0") == "1",
    )
    parts = [r["outp"] for r in res.results]
    out = np.empty((B, T, D), np.float32)
    for b in range(B):
        out[b] = parts[4 * b]
        for p in range(1, 4):
            out[b] += parts[4 * b + p]
        out[b] += bo.astype(np.float32)
    if _return_raw:
        return out, res
    return out
